# revision 15
# baseline (speedup 1.0000x reference)
"""GCN layer kernel for Trainium2, distributed over 8 NeuronCores.

Math (matches the reference, reassociated):
    out = segment_sum(X[edge_src] * edge_val, edge_dst) @ W + b
        = (A @ X) @ W + b

Distribution: 1D graph partition over destination rows. Core m owns dst rows
[m*RPC, (m+1)*RPC) and the edges that land there.

The per-edge dma_gather stream is the hard critical path (~8.6 ns/row HW
descriptor cadence for random 512B HBM reads; probed: no SWDGE queue/pipeline
configuration beats it). Two levers reduce the number of gathered rows:

1. (A@X)@W reassociation: gather bf16 X rows directly -- no phase-1 X@W, no
   support round-trip, no barriers; gathers run wall-to-wall from t=0.
2. Wide destination windows + slot dedup: a window is Q*128 dst rows with Q
   PSUM accumulators. The scaled one-hot lhsT may be MULTI-hot: edges that
   share a source within a (chunk, window) run share one gathered slot (up
   to 2 edges/slot; pair-slots are sorted first so the second one-hot layer
   only covers a tile prefix). At Q=8 this cuts gathered rows ~15% and
   shrinks run-padding (52 runs vs 392).

Per-core pipeline, per source chunk c (4 chunks bound the int16 gather
indices) and dst window (Q*128 rows):
  dma_gather pulls the run's slot sources from X_bf16 (1024 rows per call);
  DVE builds scaled one/two-hot blocks [128 slot, st*128] per 128-row half
  (int16 iota == int16 dst-local, times bf16 edge_val, built in segments of
  ST tiles); PE matmuls accumulate psum_h[128 dst, 256 din] += onehot_h.T
  @ gx for each half h; DVE adds the Q psums into a 12.8MB SBUF slab
  holding A@X for all the core's dst rows.
Epilogue per 128-row window: slab_w -> bf16 -> PE transpose halves -> GEMM
with W halves -> +bias -> out DRAM.
"""

import os
import numpy as np
import ml_dtypes

import concourse.bass as bass
import concourse.bacc as bacc
import concourse.mybir as mybir
import concourse.tile as tile
from concourse import masks
from concourse import bass_utils

F32 = mybir.dt.float32
BF16 = mybir.dt.bfloat16
I16 = mybir.dt.int16

# ---------------------------------------------------------------- config


class Cfg:
    def __init__(self, n_nodes, d, n_cores, n_chunks, gather_batch, q, st):
        self.n_nodes = n_nodes
        self.d = d                      # 256
        self.n_cores = n_cores
        self.rpc = n_nodes // n_cores   # dst rows per core
        self.n_chunks = n_chunks        # src chunks (int16 index limit)
        self.crows = n_nodes // n_chunks
        assert self.crows <= 32000
        self.gb = gather_batch          # edges per dma_gather
        assert gather_batch % 128 == 0
        self.tpg = gather_batch // 128  # tiles per gather
        self.q = q                      # 128-row halves per dst window
        self.win = q * 128
        self.nwin = (self.rpc + self.win - 1) // self.win
        self.st = st                    # one-hot build segment, in tiles
        self.nw = (self.rpc + 127) // 128   # 128-row slab windows


# gather_batch: one dma_gather pushes gb/16+1 descriptors per SWDGE ring.
# HW-probed: 1024 (65/ring) runs; 1408+ (89+/ring) wedges the device.
FULL = Cfg(n_nodes=100000, d=256, n_cores=8, n_chunks=4, gather_batch=1024,
           q=8, st=8)


# ---------------------------------------------------------------- host prep


def _preprocess(cfg, edge_src, edge_dst, edge_val):
    """Bucket edges per (core, src-chunk, dst-window); dedup same-source
    edges into pair slots (their own tile prefix per run); remaining singles
    are grouped per 128-row half. Shared tile structure: per (c,w) run =
    [K2 pair tiles][K1[h] singles tiles for h=0..q-1], padded to 128 slots
    per sub-run and to a gather multiple per chunk."""
    m_of = edge_dst // cfg.rpc
    ncw = cfg.n_chunks * cfg.nwin
    p_counts = np.zeros((cfg.n_cores, ncw), np.int64)
    s_counts = np.zeros((cfg.n_cores, ncw, cfg.q), np.int64)
    per_core = []
    for m in range(cfg.n_cores):
        sel = np.nonzero(m_of == m)[0]
        s_ = edge_src[sel]
        d_ = edge_dst[sel] - m * cfg.rpc
        v_ = edge_val[sel]
        c_ = s_ // cfg.crows
        wq = d_ // cfg.win
        dl = (d_ - wq * cfg.win).astype(np.int16)
        order = np.lexsort((s_, wq, c_))
        s_, dl, v_, c_, wq = (s_[order], dl[order], v_[order], c_[order],
                              wq[order])
        cw = c_.astype(np.int64) * cfg.nwin + wq
        runs = {}
        for u, a, b in zip(*_run_bounds(cw)):
            sr, dr, vr = s_[a:b], dl[a:b], v_[a:b]
            n = b - a
            change = np.r_[True, sr[1:] != sr[:-1]]
            first = np.nonzero(change)[0]
            gid = np.cumsum(change) - 1
            rank = np.arange(n) - first[gid]
            pos = rank % 2
            is0 = pos == 0
            slot_src = sr[is0]
            d1 = dr[is0]
            v1 = vr[is0].astype(np.float32)
            nslot = len(slot_src)
            slot_of_1 = (np.cumsum(is0) - 1)[pos == 1]
            d2 = np.zeros(nslot, np.int16)
            v2 = np.zeros(nslot, np.float32)
            d2[slot_of_1] = dr[pos == 1]
            v2[slot_of_1] = vr[pos == 1]
            haspair = np.zeros(nslot, bool)
            haspair[slot_of_1] = True
            P = (slot_src[haspair], d1[haspair], v1[haspair],
                 d2[haspair], v2[haspair])
            hi = (d1 >> 7).astype(np.int64)
            S = []
            for h in range(cfg.q):
                sh = ~haspair & (hi == h)
                S.append((slot_src[sh], d1[sh], v1[sh]))
            runs[int(u)] = (P, S)
            p_counts[m, int(u)] = len(P[0])
            for h in range(cfg.q):
                s_counts[m, int(u), h] = len(S[h][0])
        per_core.append(runs)

    K2 = (p_counts.max(axis=0) + 127) // 128              # [ncw]
    K1 = (s_counts.max(axis=0) + 127) // 128              # [ncw, q]
    ktot = K2 + K1.sum(axis=1)                            # tiles per run
    Tc = []
    for c in range(cfg.n_chunks):
        t = int(ktot[c * cfg.nwin:(c + 1) * cfg.nwin].sum())
        pad = (-t) % cfg.tpg
        K1[(c + 1) * cfg.nwin - 1, cfg.q - 1] += pad
        ktot[(c + 1) * cfg.nwin - 1] += pad
        Tc.append(t + pad)
    NT = int(sum(Tc))
    NI = NT * 128
    NT2 = int(K2.sum())

    run_start = {}
    run_start2 = {}
    t0 = 0
    t2 = 0
    for c in range(cfg.n_chunks):
        for w in range(cfg.nwin):
            u = c * cfg.nwin + w
            run_start[u] = t0 * 128
            run_start2[u] = t2 * 128
            t0 += int(ktot[u])
            t2 += int(K2[u])

    core_arrays = []
    for m in range(cfg.n_cores):
        runs = per_core[m]
        idx = np.zeros(NI, np.int16)
        dl1 = np.zeros(NI, np.int16)
        vv1 = np.zeros(NI, np.float32)
        dl2 = np.zeros(max(NT2, 1) * 128, np.int16)
        vv2 = np.zeros(max(NT2, 1) * 128, np.float32)
        for u, (P, S) in runs.items():
            c = u // cfg.nwin
            o = run_start[u]
            o2 = run_start2[u]
            n = len(P[0])
            idx[o:o + n] = (P[0] - c * cfg.crows).astype(np.int16)
            dl1[o:o + n] = P[1]
            vv1[o:o + n] = P[2]
            dl2[o2:o2 + n] = P[3]
            vv2[o2:o2 + n] = P[4]
            o += int(K2[u]) * 128
            for h in range(cfg.q):
                nh = len(S[h][0])
                idx[o:o + nh] = (S[h][0] - c * cfg.crows).astype(np.int16)
                dl1[o:o + nh] = S[h][1]
                vv1[o:o + nh] = S[h][2]
                o += int(K1[u, h]) * 128
        gidx = np.ascontiguousarray(
            np.tile(idx.reshape(NI // 16, 16).T, (8, 1)))     # [128, NI/16]
        pk = lambda a, nt, dt: np.ascontiguousarray(
            a.reshape(nt, 128).T.astype(dt)) if nt else np.zeros(
            (128, 1), dt)
        core_arrays.append((
            gidx,
            pk(dl1, NT, np.float32), pk(vv1, NT, ml_dtypes.bfloat16),
            pk(dl2, max(NT2, 1), np.float32),
            pk(vv2, max(NT2, 1), ml_dtypes.bfloat16)))
    return K2.reshape(cfg.n_chunks, cfg.nwin), \
        K1.reshape(cfg.n_chunks, cfg.nwin, cfg.q), Tc, NT, max(NT2, 1), \
        NI, core_arrays


def _run_bounds(cw):
    uniq, first = np.unique(cw, return_index=True)
    return uniq, first, list(first[1:]) + [len(cw)]


# ---------------------------------------------------------------- device IR


def _build(tc, nc, cfg, K2, K1, Tc, NT, NT2, ap):
    """Emit the per-core program (identical across cores)."""
    D = cfg.d
    n_full_w = cfg.rpc // 128
    tail_rows = cfg.rpc - n_full_w * 128

    with tc.tile_pool(name="const", bufs=1) as cp, \
         tc.tile_pool(name="slab", bufs=1) as slabp:
        w0 = cp.tile([128, D], BF16, tag="w0")
        w1 = cp.tile([128, D], BF16, tag="w1")
        nc.sync.dma_start(w0[:], ap["W"][0:128, :])
        nc.sync.dma_start(w1[:], ap["W"][128:256, :])
        bbt = cp.tile([128, D], F32, tag="bb")
        nc.sync.dma_start(bbt[:], ap["bb"][:, :])
        iotas = []
        for h in range(cfg.q):
            it = cp.tile([128, 128], F32, name=f"iota{h}", tag=f"iota{h}")
            nc.gpsimd.iota(it[:], pattern=[[1, 128]], base=h * 128,
                           channel_multiplier=0,
                           allow_small_or_imprecise_dtypes=True)
            iotas.append(it)
        identf = cp.tile([128, 128], BF16, tag="ident")
        masks.make_identity(nc, identf[:])
        dstl1 = cp.tile([128, NT], F32, tag="dstl1")
        nc.scalar.dma_start(dstl1[:], ap["dstl1"][:, :])
        val1 = cp.tile([128, NT], BF16, tag="val1")
        nc.scalar.dma_start(val1[:], ap["val1"][:, :])
        dstl2 = cp.tile([128, NT2], F32, tag="dstl2")
        nc.scalar.dma_start(dstl2[:], ap["dstl2"][:, :])
        val2 = cp.tile([128, NT2], BF16, tag="val2")
        nc.scalar.dma_start(val2[:], ap["val2"][:, :])

        # slab holds A@X (f32) for all of this core's dst rows
        slab = slabp.tile([128, cfg.nw * D], F32, tag="slab")
        nc.vector.memset(slab[:], 0.0)

        with tc.tile_pool(name="gb", bufs=5) as gbp, \
             tc.tile_pool(name="gi", bufs=8) as gip, \
             tc.tile_pool(name="oh", bufs=6) as ohp, \
             tc.tile_pool(name="ps2", bufs=1, space="PSUM") as ps2p:
            T = 0
            T2 = 0
            g_off = 0
            for c in range(cfg.n_chunks):
                xg_c = ap["Xg"][c * cfg.crows:(c + 1) * cfg.crows, :]
                n_g = Tc[c] // cfg.tpg
                gbufs = [None] * n_g
                t_in_c = 0
                t2_in_c = 0

                def need(t):
                    g = t // cfg.tpg
                    if gbufs[g] is None:
                        gb = gbp.tile([128, cfg.tpg, D], BF16, name="gb",
                                      tag="gb")
                        gi = gip.tile([128, cfg.gb // 16], I16, name="gi",
                                      tag="gi")
                        col0 = (g_off + g) * (cfg.gb // 16)
                        nc.scalar.dma_start(
                            gi[:], ap["gidx"][:, col0:col0 + cfg.gb // 16])
                        nc.gpsimd.dma_gather(
                            gb[:], xg_c, gi[:], num_idxs=cfg.gb,
                            num_idxs_reg=cfg.gb, elem_size=D)
                        gbufs[g] = gb
                    return gbufs[g][:, t % cfg.tpg, :]

                def bcast(src, t0, nt):
                    return src[:, t0:t0 + nt].rearrange(
                        "p (f o) -> p f o", o=1).broadcast_to([128, nt, 128])

                for w in range(cfg.nwin):
                    k2 = int(K2[c, w])
                    k1s = [int(K1[c, w, h]) for h in range(cfg.q)]
                    ktot = k2 + sum(k1s)
                    if ktot == 0:
                        continue
                    rows = min(cfg.win, cfg.rpc - w * cfg.win)
                    H = (rows + 127) // 128
                    T0 = T + t_in_c
                    T20 = T2 + t2_in_c
                    pss = [ps2p.tile([128, D], F32, name=f"ps{j}",
                                     tag=f"ps{j}") for j in range(H)]

                    def psh(h):
                        return pss[h][:]

                    started = [False] * cfg.q
                    # last matmul tile for each half, for the stop flag
                    lastt = [k2 - 1 if k2 else -1] * cfg.q
                    toff = k2
                    for h in range(cfg.q):
                        if k1s[h]:
                            lastt[h] = toff + k1s[h] - 1
                        toff += k1s[h]
                    for t in range(k2):
                        need(t_in_c + t)
                    # ---- pairs region: both layers, all halves ----
                    if k2:
                        for h in range(H):
                            io_b = iotas[h][:].rearrange(
                                "p (o f) -> p o f", o=1)
                            oh = ohp.tile([128, 2, k2, 128], BF16, name="ohp",
                                          tag="ohp")
                            for li, (dsrc, vsrc, Tb) in enumerate(
                                    ((dstl1, val1, T0), (dstl2, val2, T20))):
                                ohl = oh[:, li, :, :]
                                nc.vector.tensor_tensor(
                                    ohl, io_b.broadcast_to([128, k2, 128]),
                                    bcast(dsrc, Tb, k2),
                                    op=mybir.AluOpType.is_equal)
                                nc.vector.tensor_tensor(
                                    ohl, ohl, bcast(vsrc, Tb, k2),
                                    op=mybir.AluOpType.mult)
                            for t in range(k2):
                                rhs = need(t_in_c + t)
                                for li in range(2):
                                    nc.tensor.matmul(
                                        psh(h), oh[:, li, t, :], rhs,
                                        start=not started[h],
                                        stop=(t == lastt[h] and li == 1))
                                    started[h] = True
                    # ---- singles: per-half sub-runs ----
                    tof = k2
                    for h in range(cfg.q):
                        k1 = k1s[h]
                        if k1 == 0:
                            continue
                        hh = min(h, H - 1)
                        io_b = iotas[hh][:].rearrange("p (o f) -> p o f", o=1)
                        oh = ohp.tile([128, k1, 128], BF16, name="ohs",
                                      tag="ohs")
                        nc.vector.tensor_tensor(
                            oh[:], io_b.broadcast_to([128, k1, 128]),
                            bcast(dstl1, T0 + tof, k1),
                            op=mybir.AluOpType.is_equal)
                        nc.vector.tensor_tensor(
                            oh[:], oh[:], bcast(val1, T0 + tof, k1),
                            op=mybir.AluOpType.mult)
                        for t in range(k1):
                            rhs = need(t_in_c + tof + t)
                            nc.tensor.matmul(
                                psh(hh), oh[:, t, :], rhs,
                                start=not started[hh],
                                stop=(tof + t == lastt[hh]))
                            started[hh] = True
                        tof += k1
                    t_in_c += ktot
                    t2_in_c += k2
                    for h in range(H):
                        if not started[h]:
                            continue
                        wslab = w * cfg.q + h
                        sl = slab[:, wslab * D:(wslab + 1) * D]
                        nc.vector.tensor_tensor(sl, sl, psh(h),
                                                op=mybir.AluOpType.add)
                T += Tc[c]
                T2 += int(K2[c].sum())
                g_off += Tc[c] // cfg.tpg

        # ---------------- epilogue: out_w = slab_w @ W + b ----------------
        with tc.tile_pool(name="est", bufs=3) as estp, \
             tc.tile_pool(name="etr", bufs=4) as etrp, \
             tc.tile_pool(name="eps", bufs=2, space="PSUM") as epsp, \
             tc.tile_pool(name="ops", bufs=2, space="PSUM") as opsp, \
             tc.tile_pool(name="eout", bufs=3) as eoutp:
            for w in range(cfg.nw):
                sl = slab[:, w * D:(w + 1) * D]
                stg = estp.tile([128, D], BF16, tag="stg")
                nc.scalar.copy(stg[:], sl)
                ops = opsp.tile([128, D], F32, tag="ops")
                for h in range(2):
                    trp = epsp.tile([128, 128], BF16, tag="trp")
                    nc.tensor.transpose(
                        out=trp[:], in_=stg[:, h * 128:(h + 1) * 128],
                        identity=identf[:])
                    trs = etrp.tile([128, 128], BF16, tag="trs")
                    nc.scalar.copy(trs[:], trp[:])
                    nc.tensor.matmul(ops[:], trs[:], (w0 if h == 0 else w1)[:],
                                     start=(h == 0), stop=(h == 1))
                outt = eoutp.tile([128, D], F32, tag="outt")
                nc.vector.tensor_tensor(outt[:], ops[:], bbt[:],
                                        op=mybir.AluOpType.add)
                rows = 128 if w < cfg.nw - 1 or tail_rows == 0 else tail_rows
                nc.sync.dma_start(
                    ap["out"][w * 128:w * 128 + rows, :], outt[0:rows, :])


def build_program(cfg, K2, K1, Tc, NT, NT2, NI, debug=False):
    nc = bacc.Bacc("TRN2", target_bir_lowering=False, debug=debug,
                   enable_asserts=False, num_devices=cfg.n_cores)
    ap = {
        "Xg": nc.dram_tensor("Xg", [cfg.n_nodes, cfg.d], BF16,
                             kind="ExternalInput").ap(),
        "W": nc.dram_tensor("W", [cfg.d, cfg.d], BF16,
                            kind="ExternalInput").ap(),
        "bb": nc.dram_tensor("bb", [128, cfg.d], F32,
                             kind="ExternalInput").ap(),
        "gidx": nc.dram_tensor("gidx", [128, NI // 16], I16,
                               kind="ExternalInput").ap(),
        "dstl1": nc.dram_tensor("dstl1", [128, NT], F32,
                                kind="ExternalInput").ap(),
        "val1": nc.dram_tensor("val1", [128, NT], BF16,
                               kind="ExternalInput").ap(),
        "dstl2": nc.dram_tensor("dstl2", [128, NT2], F32,
                                kind="ExternalInput").ap(),
        "val2": nc.dram_tensor("val2", [128, NT2], BF16,
                               kind="ExternalInput").ap(),
        "out": nc.dram_tensor("out", [cfg.rpc, cfg.d], F32,
                              kind="ExternalOutput").ap(),
    }
    with tile.TileContext(nc) as tc:
        _build(tc, nc, cfg, K2, K1, Tc, NT, NT2, ap)
    nc.compile()
    return nc


# ---------------------------------------------------------------- entry


last_run_info = {}


def kernel(X, edge_src, edge_dst, edge_val, W, b):
    cfg = FULL
    X = np.asarray(X, np.float32)
    W = np.asarray(W, np.float32)
    b = np.asarray(b, np.float32)
    edge_src = np.asarray(edge_src, np.int32)
    edge_dst = np.asarray(edge_dst, np.int32)
    edge_val = np.asarray(edge_val, np.float32)

    K2, K1, Tc, NT, NT2, NI, core_arrays = _preprocess(
        cfg, edge_src, edge_dst, edge_val)
    nc = build_program(cfg, K2, K1, Tc, NT, NT2, NI)

    Xg = np.ascontiguousarray(X.astype(ml_dtypes.bfloat16))
    Wb = np.ascontiguousarray(W.astype(ml_dtypes.bfloat16))
    bb = np.ascontiguousarray(np.broadcast_to(b, (128, cfg.d))).astype(
        np.float32)
    in_maps = []
    for m in range(cfg.n_cores):
        gidx, d1, v1, d2, v2 = core_arrays[m]
        in_maps.append({"Xg": Xg, "W": Wb, "bb": bb, "gidx": gidx,
                        "dstl1": d1, "val1": v1, "dstl2": d2, "val2": v2})

    trace = bool(int(os.environ.get("GCN_TRACE", "0")))
    res = bass_utils.run_bass_kernel_spmd(
        nc, in_maps, core_ids=list(range(cfg.n_cores)), trace=trace)
    last_run_info.clear()
    last_run_info.update(exec_time_ns=res.exec_time_ns,
                         profile_json=res.profile_json)

    out = np.concatenate([res.results[m]["out"] for m in range(cfg.n_cores)],
                         axis=0)
    return out


# revision 16
# speedup vs baseline: 1.2627x; 1.2627x over previous
"""GCN layer kernel for Trainium2, distributed over 8 NeuronCores.

Math (matches the reference, reassociated):
    out = segment_sum(X[edge_src] * edge_val, edge_dst) @ W + b
        = (A @ X) @ W + b

Distribution: 1D graph partition over destination rows. Core m owns dst rows
[m*RPC, (m+1)*RPC) and the edges that land there.

The per-edge dma_gather stream is the hard critical path (~8.6 ns/row HW
descriptor cadence for random 512B HBM reads; probed: no SWDGE queue/pipeline
configuration beats it). Two levers reduce the number of gathered rows:

1. (A@X)@W reassociation: gather bf16 X rows directly -- no phase-1 X@W, no
   support round-trip, no barriers; gathers run wall-to-wall from t=0.
2. Wide destination windows + slot dedup: a window is Q*128 dst rows with Q
   PSUM accumulators. The scaled one-hot lhsT may be MULTI-hot: edges that
   share a source within a (chunk, window) run share one gathered slot (up
   to 2 edges/slot; pair-slots are sorted first so the second one-hot layer
   only covers a tile prefix). At Q=8 this cuts gathered rows ~15% and
   shrinks run-padding (52 runs vs 392).

Per-core pipeline, per source chunk c (4 chunks bound the int16 gather
indices) and dst window (Q*128 rows):
  dma_gather pulls the run's slot sources from X_bf16 (1024 rows per call);
  DVE builds scaled one/two-hot blocks [128 slot, st*128] per 128-row half
  (int16 iota == int16 dst-local, times bf16 edge_val, built in segments of
  ST tiles); PE matmuls accumulate psum_h[128 dst, 256 din] += onehot_h.T
  @ gx for each half h; DVE adds the Q psums into a 12.8MB SBUF slab
  holding A@X for all the core's dst rows.
Epilogue per 128-row window: slab_w -> bf16 -> PE transpose halves -> GEMM
with W halves -> +bias -> out DRAM.
"""

import os
import numpy as np
import ml_dtypes

import concourse.bass as bass
import concourse.bacc as bacc
import concourse.mybir as mybir
import concourse.tile as tile
from concourse import masks
from concourse import bass_utils

F32 = mybir.dt.float32
BF16 = mybir.dt.bfloat16
I16 = mybir.dt.int16

# ---------------------------------------------------------------- config


class Cfg:
    def __init__(self, n_nodes, d, n_cores, n_chunks, gather_batch, q, st):
        self.n_nodes = n_nodes
        self.d = d                      # 256
        self.n_cores = n_cores
        self.rpc = n_nodes // n_cores   # dst rows per core
        self.n_chunks = n_chunks        # src chunks (int16 index limit)
        self.crows = n_nodes // n_chunks
        assert self.crows <= 32000
        self.gb = gather_batch          # edges per dma_gather
        assert gather_batch % 128 == 0
        self.tpg = gather_batch // 128  # tiles per gather
        self.q = q                      # 128-row halves per dst window
        self.win = q * 128
        self.nwin = (self.rpc + self.win - 1) // self.win
        self.st = st                    # one-hot build segment, in tiles
        self.nw = (self.rpc + 127) // 128   # 128-row slab windows


# gather_batch: one dma_gather pushes gb/16+1 descriptors per SWDGE ring.
# HW-probed: 1024 (65/ring) runs; 1408+ (89+/ring) wedges the device.
FULL = Cfg(n_nodes=100000, d=256, n_cores=8, n_chunks=4, gather_batch=1024,
           q=8, st=8)


# ---------------------------------------------------------------- host prep


def _preprocess(cfg, edge_src, edge_dst, edge_val):
    """Bucket edges per (core, src-chunk, dst-window); dedup same-source
    edges into pair slots (their own tile prefix per run); remaining singles
    are grouped per 128-row half. Shared tile structure: per (c,w) run =
    [K2 pair tiles][K1[h] singles tiles for h=0..q-1], padded to 128 slots
    per sub-run and to a gather multiple per chunk."""
    m_of = edge_dst // cfg.rpc
    ncw = cfg.n_chunks * cfg.nwin
    p_counts = np.zeros((cfg.n_cores, ncw), np.int64)
    s_counts = np.zeros((cfg.n_cores, ncw, cfg.q), np.int64)
    per_core = []
    for m in range(cfg.n_cores):
        sel = np.nonzero(m_of == m)[0]
        s_ = edge_src[sel]
        d_ = edge_dst[sel] - m * cfg.rpc
        v_ = edge_val[sel]
        c_ = s_ // cfg.crows
        wq = d_ // cfg.win
        dl = (d_ - wq * cfg.win).astype(np.int16)
        order = np.lexsort((s_, wq, c_))
        s_, dl, v_, c_, wq = (s_[order], dl[order], v_[order], c_[order],
                              wq[order])
        cw = c_.astype(np.int64) * cfg.nwin + wq
        runs = {}
        for u, a, b in zip(*_run_bounds(cw)):
            sr, dr, vr = s_[a:b], dl[a:b], v_[a:b]
            n = b - a
            change = np.r_[True, sr[1:] != sr[:-1]]
            first = np.nonzero(change)[0]
            gid = np.cumsum(change) - 1
            rank = np.arange(n) - first[gid]
            pos = rank % 2
            is0 = pos == 0
            slot_src = sr[is0]
            d1 = dr[is0]
            v1 = vr[is0].astype(np.float32)
            nslot = len(slot_src)
            slot_of_1 = (np.cumsum(is0) - 1)[pos == 1]
            d2 = np.zeros(nslot, np.int16)
            v2 = np.zeros(nslot, np.float32)
            d2[slot_of_1] = dr[pos == 1]
            v2[slot_of_1] = vr[pos == 1]
            haspair = np.zeros(nslot, bool)
            haspair[slot_of_1] = True
            P = (slot_src[haspair], d1[haspair], v1[haspair],
                 d2[haspair], v2[haspair])
            hi = (d1 >> 7).astype(np.int64)
            S = []
            for h in range(cfg.q):
                sh = ~haspair & (hi == h)
                S.append((slot_src[sh], d1[sh], v1[sh]))
            runs[int(u)] = (P, S)
            p_counts[m, int(u)] = len(P[0])
            for h in range(cfg.q):
                s_counts[m, int(u), h] = len(S[h][0])
        per_core.append(runs)

    K2 = (p_counts.max(axis=0) + 127) // 128              # [ncw]
    K1 = (s_counts.max(axis=0) + 127) // 128              # [ncw, q]
    ktot = K2 + K1.sum(axis=1)                            # tiles per run
    Tc = []
    for c in range(cfg.n_chunks):
        t = int(ktot[c * cfg.nwin:(c + 1) * cfg.nwin].sum())
        pad = (-t) % cfg.tpg
        K1[(c + 1) * cfg.nwin - 1, cfg.q - 1] += pad
        ktot[(c + 1) * cfg.nwin - 1] += pad
        Tc.append(t + pad)
    NT = int(sum(Tc))
    NI = NT * 128
    NT2 = int(K2.sum())

    run_start = {}
    run_start2 = {}
    t0 = 0
    t2 = 0
    for c in range(cfg.n_chunks):
        for w in range(cfg.nwin):
            u = c * cfg.nwin + w
            run_start[u] = t0 * 128
            run_start2[u] = t2 * 128
            t0 += int(ktot[u])
            t2 += int(K2[u])

    core_arrays = []
    for m in range(cfg.n_cores):
        runs = per_core[m]
        idx = np.zeros(NI, np.int16)
        dl1 = np.zeros(NI, np.int16)
        vv1 = np.zeros(NI, np.float32)
        dl2 = np.zeros(max(NT2, 1) * 128, np.int16)
        vv2 = np.zeros(max(NT2, 1) * 128, np.float32)
        for u, (P, S) in runs.items():
            c = u // cfg.nwin
            o = run_start[u]
            o2 = run_start2[u]
            n = len(P[0])
            idx[o:o + n] = (P[0] - c * cfg.crows).astype(np.int16)
            dl1[o:o + n] = P[1]
            vv1[o:o + n] = P[2]
            dl2[o2:o2 + n] = P[3]
            vv2[o2:o2 + n] = P[4]
            o += int(K2[u]) * 128
            for h in range(cfg.q):
                nh = len(S[h][0])
                idx[o:o + nh] = (S[h][0] - c * cfg.crows).astype(np.int16)
                dl1[o:o + nh] = S[h][1]
                vv1[o:o + nh] = S[h][2]
                o += int(K1[u, h]) * 128
        gidx = np.ascontiguousarray(
            np.tile(idx.reshape(NI // 16, 16).T, (8, 1)))     # [128, NI/16]
        pk = lambda a, nt, dt: np.ascontiguousarray(
            a.reshape(nt, 128).T.astype(dt)) if nt else np.zeros(
            (128, 1), dt)
        core_arrays.append((
            gidx,
            pk(dl1, NT, np.float32), pk(vv1, NT, ml_dtypes.bfloat16),
            pk(dl2, max(NT2, 1), np.float32),
            pk(vv2, max(NT2, 1), ml_dtypes.bfloat16)))
    return K2.reshape(cfg.n_chunks, cfg.nwin), \
        K1.reshape(cfg.n_chunks, cfg.nwin, cfg.q), Tc, NT, max(NT2, 1), \
        NI, core_arrays


def _run_bounds(cw):
    uniq, first = np.unique(cw, return_index=True)
    return uniq, first, list(first[1:]) + [len(cw)]


# ---------------------------------------------------------------- device IR


def _build(tc, nc, cfg, K2, K1, Tc, NT, NT2, ap):
    """Emit the per-core program (identical across cores)."""
    D = cfg.d
    n_full_w = cfg.rpc // 128
    tail_rows = cfg.rpc - n_full_w * 128

    with tc.tile_pool(name="const", bufs=1) as cp, \
         tc.tile_pool(name="slab", bufs=1) as slabp:
        w0 = cp.tile([128, D], BF16, tag="w0")
        w1 = cp.tile([128, D], BF16, tag="w1")
        nc.sync.dma_start(w0[:], ap["W"][0:128, :])
        nc.sync.dma_start(w1[:], ap["W"][128:256, :])
        bbt = cp.tile([128, D], F32, tag="bb")
        nc.sync.dma_start(bbt[:], ap["bb"][:, :])
        iotas = []
        for h in range(cfg.q):
            it = cp.tile([128, 128], F32, name=f"iota{h}", tag=f"iota{h}")
            nc.gpsimd.iota(it[:], pattern=[[1, 128]], base=h * 128,
                           channel_multiplier=0,
                           allow_small_or_imprecise_dtypes=True)
            iotas.append(it)
        identf = cp.tile([128, 128], BF16, tag="ident")
        masks.make_identity(nc, identf[:])
        dstl1 = cp.tile([128, NT], F32, tag="dstl1")
        nc.scalar.dma_start(dstl1[:], ap["dstl1"][:, :])
        val1 = cp.tile([128, NT], BF16, tag="val1")
        nc.scalar.dma_start(val1[:], ap["val1"][:, :])
        dstl2 = cp.tile([128, NT2], F32, tag="dstl2")
        nc.scalar.dma_start(dstl2[:], ap["dstl2"][:, :])
        val2 = cp.tile([128, NT2], BF16, tag="val2")
        nc.scalar.dma_start(val2[:], ap["val2"][:, :])

        # slab holds A@X (f32) for all of this core's dst rows
        slab = slabp.tile([128, cfg.nw * D], F32, tag="slab")
        nc.vector.memset(slab[:], 0.0)

        with tc.tile_pool(name="gb", bufs=10) as gbp, \
             tc.tile_pool(name="gi", bufs=8) as gip, \
             tc.tile_pool(name="oh", bufs=4) as ohp, \
             tc.tile_pool(name="ps2", bufs=1, space="PSUM") as ps2p:
            T = 0
            T2 = 0
            g_off = 0
            for c in range(cfg.n_chunks):
                xg_c = ap["Xg"][c * cfg.crows:(c + 1) * cfg.crows, :]
                n_g = Tc[c] // cfg.tpg
                gbufs = [None] * n_g
                t_in_c = 0
                t2_in_c = 0

                def need(t):
                    g = t // cfg.tpg
                    if gbufs[g] is None:
                        gb = gbp.tile([128, cfg.tpg, D], BF16, name="gb",
                                      tag="gb")
                        gi = gip.tile([128, cfg.gb // 16], I16, name="gi",
                                      tag="gi")
                        col0 = (g_off + g) * (cfg.gb // 16)
                        nc.scalar.dma_start(
                            gi[:], ap["gidx"][:, col0:col0 + cfg.gb // 16])
                        nc.gpsimd.dma_gather(
                            gb[:], xg_c, gi[:], num_idxs=cfg.gb,
                            num_idxs_reg=cfg.gb, elem_size=D)
                        gbufs[g] = gb
                    return gbufs[g][:, t % cfg.tpg, :]

                def bcast(src, t0, nt):
                    return src[:, t0:t0 + nt].rearrange(
                        "p (f o) -> p f o", o=1).broadcast_to([128, nt, 128])

                for w in range(cfg.nwin):
                    k2 = int(K2[c, w])
                    k1s = [int(K1[c, w, h]) for h in range(cfg.q)]
                    ktot = k2 + sum(k1s)
                    if ktot == 0:
                        continue
                    rows = min(cfg.win, cfg.rpc - w * cfg.win)
                    H = (rows + 127) // 128
                    T0 = T + t_in_c
                    T20 = T2 + t2_in_c
                    pss = [ps2p.tile([128, D], F32, name=f"ps{j}",
                                     tag=f"ps{j}") for j in range(H)]

                    def psh(h):
                        return pss[h][:]

                    started = [False] * cfg.q
                    # last matmul tile for each half, for the stop flag
                    lastt = [k2 - 1 if k2 else -1] * cfg.q
                    toff = k2
                    for h in range(cfg.q):
                        if k1s[h]:
                            lastt[h] = toff + k1s[h] - 1
                        toff += k1s[h]
                    for t in range(k2):
                        need(t_in_c + t)
                    # ---- pairs region: both layers, all halves ----
                    if k2:
                        for h in range(H):
                            io_b = iotas[h][:].rearrange(
                                "p (o f) -> p o f", o=1)
                            oh = ohp.tile([128, 2, k2, 128], BF16, name="ohp",
                                          tag="ohp")
                            for li, (dsrc, vsrc, Tb) in enumerate(
                                    ((dstl1, val1, T0), (dstl2, val2, T20))):
                                ohl = oh[:, li, :, :]
                                nc.vector.tensor_tensor(
                                    ohl, io_b.broadcast_to([128, k2, 128]),
                                    bcast(dsrc, Tb, k2),
                                    op=mybir.AluOpType.is_equal)
                                nc.vector.tensor_tensor(
                                    ohl, ohl, bcast(vsrc, Tb, k2),
                                    op=mybir.AluOpType.mult)
                            for t in range(k2):
                                rhs = need(t_in_c + t)
                                for li in range(2):
                                    nc.tensor.matmul(
                                        psh(h), oh[:, li, t, :], rhs,
                                        start=not started[h],
                                        stop=(t == lastt[h] and li == 1))
                                    started[h] = True
                    # ---- singles: per-half sub-runs ----
                    tof = k2
                    for h in range(cfg.q):
                        k1 = k1s[h]
                        if k1 == 0:
                            continue
                        hh = min(h, H - 1)
                        io_b = iotas[hh][:].rearrange("p (o f) -> p o f", o=1)
                        oh = ohp.tile([128, k1, 128], BF16, name="ohs",
                                      tag="ohs")
                        nc.vector.tensor_tensor(
                            oh[:], io_b.broadcast_to([128, k1, 128]),
                            bcast(dstl1, T0 + tof, k1),
                            op=mybir.AluOpType.is_equal)
                        nc.vector.tensor_tensor(
                            oh[:], oh[:], bcast(val1, T0 + tof, k1),
                            op=mybir.AluOpType.mult)
                        for t in range(k1):
                            rhs = need(t_in_c + tof + t)
                            nc.tensor.matmul(
                                psh(hh), oh[:, t, :], rhs,
                                start=not started[hh],
                                stop=(tof + t == lastt[hh]))
                            started[hh] = True
                        tof += k1
                    t_in_c += ktot
                    t2_in_c += k2
                    for h in range(H):
                        if not started[h]:
                            continue
                        wslab = w * cfg.q + h
                        sl = slab[:, wslab * D:(wslab + 1) * D]
                        nc.vector.tensor_tensor(sl, sl, psh(h),
                                                op=mybir.AluOpType.add)
                T += Tc[c]
                T2 += int(K2[c].sum())
                g_off += Tc[c] // cfg.tpg

        # ---------------- epilogue: out_w = slab_w @ W + b ----------------
        with tc.tile_pool(name="est", bufs=3) as estp, \
             tc.tile_pool(name="etr", bufs=4) as etrp, \
             tc.tile_pool(name="eps", bufs=2, space="PSUM") as epsp, \
             tc.tile_pool(name="ops", bufs=2, space="PSUM") as opsp, \
             tc.tile_pool(name="eout", bufs=3) as eoutp:
            for w in range(cfg.nw):
                sl = slab[:, w * D:(w + 1) * D]
                stg = estp.tile([128, D], BF16, tag="stg")
                nc.scalar.copy(stg[:], sl)
                ops = opsp.tile([128, D], F32, tag="ops")
                for h in range(2):
                    trp = epsp.tile([128, 128], BF16, tag="trp")
                    nc.tensor.transpose(
                        out=trp[:], in_=stg[:, h * 128:(h + 1) * 128],
                        identity=identf[:])
                    trs = etrp.tile([128, 128], BF16, tag="trs")
                    nc.scalar.copy(trs[:], trp[:])
                    nc.tensor.matmul(ops[:], trs[:], (w0 if h == 0 else w1)[:],
                                     start=(h == 0), stop=(h == 1))
                outt = eoutp.tile([128, D], F32, tag="outt")
                nc.vector.tensor_tensor(outt[:], ops[:], bbt[:],
                                        op=mybir.AluOpType.add)
                rows = 128 if w < cfg.nw - 1 or tail_rows == 0 else tail_rows
                nc.sync.dma_start(
                    ap["out"][w * 128:w * 128 + rows, :], outt[0:rows, :])


def build_program(cfg, K2, K1, Tc, NT, NT2, NI, debug=False):
    nc = bacc.Bacc("TRN2", target_bir_lowering=False, debug=debug,
                   enable_asserts=False, num_devices=cfg.n_cores)
    ap = {
        "Xg": nc.dram_tensor("Xg", [cfg.n_nodes, cfg.d], BF16,
                             kind="ExternalInput").ap(),
        "W": nc.dram_tensor("W", [cfg.d, cfg.d], BF16,
                            kind="ExternalInput").ap(),
        "bb": nc.dram_tensor("bb", [128, cfg.d], F32,
                             kind="ExternalInput").ap(),
        "gidx": nc.dram_tensor("gidx", [128, NI // 16], I16,
                               kind="ExternalInput").ap(),
        "dstl1": nc.dram_tensor("dstl1", [128, NT], F32,
                                kind="ExternalInput").ap(),
        "val1": nc.dram_tensor("val1", [128, NT], BF16,
                               kind="ExternalInput").ap(),
        "dstl2": nc.dram_tensor("dstl2", [128, NT2], F32,
                                kind="ExternalInput").ap(),
        "val2": nc.dram_tensor("val2", [128, NT2], BF16,
                               kind="ExternalInput").ap(),
        "out": nc.dram_tensor("out", [cfg.rpc, cfg.d], F32,
                              kind="ExternalOutput").ap(),
    }
    with tile.TileContext(nc) as tc:
        _build(tc, nc, cfg, K2, K1, Tc, NT, NT2, ap)
    nc.compile()
    return nc


# ---------------------------------------------------------------- entry


last_run_info = {}


def kernel(X, edge_src, edge_dst, edge_val, W, b):
    cfg = FULL
    X = np.asarray(X, np.float32)
    W = np.asarray(W, np.float32)
    b = np.asarray(b, np.float32)
    edge_src = np.asarray(edge_src, np.int32)
    edge_dst = np.asarray(edge_dst, np.int32)
    edge_val = np.asarray(edge_val, np.float32)

    K2, K1, Tc, NT, NT2, NI, core_arrays = _preprocess(
        cfg, edge_src, edge_dst, edge_val)
    nc = build_program(cfg, K2, K1, Tc, NT, NT2, NI)

    Xg = np.ascontiguousarray(X.astype(ml_dtypes.bfloat16))
    Wb = np.ascontiguousarray(W.astype(ml_dtypes.bfloat16))
    bb = np.ascontiguousarray(np.broadcast_to(b, (128, cfg.d))).astype(
        np.float32)
    in_maps = []
    for m in range(cfg.n_cores):
        gidx, d1, v1, d2, v2 = core_arrays[m]
        in_maps.append({"Xg": Xg, "W": Wb, "bb": bb, "gidx": gidx,
                        "dstl1": d1, "val1": v1, "dstl2": d2, "val2": v2})

    trace = bool(int(os.environ.get("GCN_TRACE", "0")))
    res = bass_utils.run_bass_kernel_spmd(
        nc, in_maps, core_ids=list(range(cfg.n_cores)), trace=trace)
    last_run_info.clear()
    last_run_info.update(exec_time_ns=res.exec_time_ns,
                         profile_json=res.profile_json)

    out = np.concatenate([res.results[m]["out"] for m in range(cfg.n_cores)],
                         axis=0)
    return out
